# revision 21
# baseline (speedup 1.0000x reference)
"""Trainium2 Bass kernel for nn_CrossAttention_47004122087816.

Math (faithful to the reference's "buggy einsum"):
    xn   = LayerNorm(x); xnb = xn * ln_w + ln_b
    q    = (xnb @ Wq) * SCALE            [n, E]
    k, v = split(media @ Wkv)            [m, E] each
    sim  = q @ k^T                       [n, m]
    colsum[j] = sum_i softmax(sim, -1)[i, j]
    out[j, :] = colsum[j] * (v @ Wout)[j, :]

Sharding: pure data-parallel - batch b=8 over 8 NeuronCores.

v2 redesign (vs the DMA-transpose baseline):
 - x and media are transposed on the HOST (layout-only prep, like the
   existing weight-row permutation), so the device loads land directly in
   the [D-part, rows-free] layout every matmul wants.  This removes all 32
   on-chip dma_start_transpose ops (~8MB of DMA traffic and the 35us
   startup serialization they caused).
 - LayerNorm is restructured to work in the transposed layout:
     q_i = r_i * (x_i @ wq' - mu_i * colsum(wq') + sigma_i * q0)
   with wq' = Wq * ln_w * SCALE, q0 = SCALE * ln_b @ Wq.  Sx and Sxx come
   from ones-vector matmuls (cheap M=1 PE work), the rank-1 corrections are
   K=1 matmuls accumulated into the q PSUM groups, and the final per-row
   scale r_i is folded into the sim-phase Exp activation's per-partition
   `scale` operand - zero extra elementwise passes over q.
 - v is never materialized: W2 = Wkv_v @ Wout is folded on the host, and
   Y = media @ W2 is computed directly (same FLOPs, one less PSUM
   evacuation pass and 2MB less SBUF).
 - sigma rows are flipped to per-partition columns with 16 tiny PE
   transposes (rhs = 1x1 identity); same trick turns the colsum PSUM rows
   into per-partition scalars in the tail, replacing 16 single-column
   scatter DMAs.
 - fp8 was evaluated (DoubleRow would halve PE time) and rejected: exp()
   amplification puts even k-only fp8 at ~2e-2 rel err, the whole gate.
 - Tail: colsum transposes -> one [128,16] copy -> 16 scaled copies
   (DVE/ScalarE alternating) -> paired 4KB-descriptor SWDGE stores.
"""

import sys

for _p in ("/opt/trn_rl_repo",):
    if _p not in sys.path:
        sys.path.insert(0, _p)

import numpy as np
import ml_dtypes

import concourse.bass as bass  # noqa: F401
import concourse.tile as tile
from concourse import bacc, mybir
from concourse.bass_utils import run_bass_kernel_spmd

B = 8
N = 2048          # x rows per batch element
M = 2048          # media rows per batch element
D = 1024          # model dim
E = 512           # inner dim
P = 128           # partitions
F = 512           # one PSUM bank of fp32
KT = D // P       # 8  contraction tiles over model dim
ET = E // P       # 4  contraction tiles over inner dim
NT = N // P       # 16 row tiles (positions)
JC = M // F       # 4  column chunks of 512
CH = N // F       # 4  position chunks of 512
SCALE = 64 ** -0.5
EPS = 1e-5

FP = mybir.dt.float32
BF = mybir.dt.bfloat16

AF = mybir.ActivationFunctionType
ALU = mybir.AluOpType
AX = mybir.AxisListType


def _build():
    nc = bacc.Bacc("TRN2", target_bir_lowering=False, debug=False, num_devices=B)

    # host layouts (see _run): xt/mt row (c*1024 + p*8 + kt) col i' holds
    # x[c*512 + i', kt*128 + p] -> per-partition 8KB contiguous loads.
    xt = nc.dram_tensor("xt", [CH * D, F], BF, kind="ExternalInput").ap()
    mt = nc.dram_tensor("mt", [CH * D, F], BF, kind="ExternalInput").ap()
    wq = nc.dram_tensor("wq", [D, E], BF, kind="ExternalInput").ap()
    wk = nc.dram_tensor("wk", [D, E], BF, kind="ExternalInput").ap()
    w2 = nc.dram_tensor("w2", [D, D], BF, kind="ExternalInput").ap()
    wqq = nc.dram_tensor("wqq", [2, E], BF, kind="ExternalInput").ap()
    out = nc.dram_tensor("out", [M, D], BF, kind="ExternalOutput").ap()

    xtv = xt.rearrange("(c p kt) i -> c p kt i", p=P, kt=KT)
    mtv = mt.rearrange("(c p kt) i -> c p kt i", p=P, kt=KT)
    # store tile jt partition p -> HBM row p*16+jt (host unscrambles);
    # paired stores give 4KB contiguous per-partition descriptors.
    ov = out.rearrange("(p t) d -> p t d", t=NT)

    with tile.TileContext(nc) as tc:
        from contextlib import ExitStack

        with ExitStack() as ctx:
            consts = ctx.enter_context(tc.tile_pool(name="consts", bufs=1))
            acts = ctx.enter_context(tc.tile_pool(name="acts", bufs=1))
            mtp = ctx.enter_context(tc.tile_pool(name="mtp", bufs=4))
            xtp = ctx.enter_context(tc.tile_pool(name="xtp", bufs=2))
            sqp = ctx.enter_context(tc.tile_pool(name="sqp", bufs=1))
            rows = ctx.enter_context(tc.tile_pool(name="rows", bufs=2))
            expp = ctx.enter_context(tc.tile_pool(name="expp", bufs=2))
            zsp = ctx.enter_context(tc.tile_pool(name="zsp", bufs=2))
            zrbp = ctx.enter_context(tc.tile_pool(name="zrbp", bufs=2))
            obuf = ctx.enter_context(tc.tile_pool(name="obuf", bufs=4))
            pmm = ctx.enter_context(tc.tile_pool(name="pmm", bufs=3, space="PSUM"))
            pyy = ctx.enter_context(tc.tile_pool(name="pyy", bufs=1, space="PSUM"))
            pst = ctx.enter_context(tc.tile_pool(name="pst", bufs=1, space="PSUM"))
            ptp = ctx.enter_context(tc.tile_pool(name="ptp", bufs=1, space="PSUM"))
            pcs = ctx.enter_context(tc.tile_pool(name="pcs", bufs=1, space="PSUM"))

            wq_t = consts.tile([P, KT, E], BF)
            wk_t = consts.tile([P, KT, E], BF)
            w2_t = consts.tile([P, KT, D], BF)
            wqq_t = consts.tile([2, E], BF)
            wdum = consts.tile([P, F], BF)
            ones_t = consts.tile([P, 1], BF)
            idf = consts.tile([P, 1], FP)     # 1x1 identity slices for transposes
            eps_t = consts.tile([1, 1], FP)
            r_sb = consts.tile([P, NT], FP)   # 1/sigma per position column
            colsb = consts.tile([P, NT], FP)
            csum_sb = consts.tile([P, F], FP)

            kT = acts.tile([P, ET, M], BF)
            qT = acts.tile([P, ET, N], BF)
            Y = acts.tile([P, NT, D], BF)

            nc.vector.memset(ones_t[:], 1.0)
            nc.vector.memset(idf[:], 1.0)
            nc.vector.memset(eps_t[:], EPS)
            nc.vector.memset(wdum[:], 0.125)
            # PE p-state warm-up during the initial DMA window
            for wdi in range(5):
                pd = pmm.tile([P, F], FP, tag="ps", name=f"warm{wdi}")
                nc.tensor.matmul(
                    pd[:], lhsT=wdum[:, 0:P], rhs=wdum[:], start=True, stop=True
                )

            # ---------------- bulk loads -------------------------------------
            mts: list = []
            xts: list = []

            def load_m(c):
                t = mtp.tile([P, KT, F], BF, tag="mt", name=f"mt{c}")
                nc.gpsimd.dma_start(t[:], mtv[c])
                mts.append(t)

            def load_x(c):
                t = xtp.tile([P, KT, F], BF, tag="xt", name=f"xt{c}")
                nc.gpsimd.dma_start(t[:], xtv[c])
                xts.append(t)

            # SWDGE in need-order; wk/wq/wqq on scalar HWDGE; w2 on sync,
            # emitted after k_chunk(0) so k0's queue-semaphore wait
            # does not cover it.
            wkv = wk.rearrange("(p kt) e -> p kt e", kt=KT)
            nc.scalar.dma_start(wk_t[:, 0 : KT // 2, :], wkv[:, 0 : KT // 2, :])
            nc.scalar.dma_start(wk_t[:, KT // 2 :, :], wkv[:, KT // 2 :, :])
            # mt0 halves stream in PARALLEL: first half on the sync HWDGE
            # queue (live from ~8.7us), second half as the first SWDGE item
            # (live from ~12us) - k0's 16 half-a matmuls cover the gap.
            t0 = mtp.tile([P, KT, F], BF, tag="mt", name="mt0")
            nc.sync.dma_start(t0[:, 0 : KT // 2, :], mtv[0][:, 0 : KT // 2, :])
            nc.gpsimd.dma_start(t0[:, KT // 2 :, :], mtv[0][:, KT // 2 :, :])
            mts.append(t0)
            load_x(0)
            load_m(1)
            load_x(1)
            load_m(2)
            load_x(2)
            load_m(3)
            load_x(3)
            # w2 last on SWDGE: streams only after all x/media loads, so it
            # never competes with feed-critical traffic; ready well before
            # the first y_group.
            nc.gpsimd.dma_start(
                w2_t[:], w2.rearrange("(p kt) d -> p kt d", kt=KT)
            )
            # ---------------- feed helpers -----------------------------------
            def k_chunk(c):
                for e in range(ET):
                    ps = pmm.tile([P, F], FP, tag="ps", name=f"k{c}_{e}")
                    for kt in range(KT):
                        nc.tensor.matmul(
                            ps[:],
                            lhsT=wk_t[:, kt, e * P : (e + 1) * P],
                            rhs=mts[c][:, kt, :],
                            start=(kt == 0),
                            stop=(kt == KT - 1),
                        )
                    dst = kT[:, e, c * F : (c + 1) * F]
                    if e % 2 == 0:
                        nc.scalar.copy(dst, ps[:])
                    else:
                        nc.vector.tensor_copy(dst, ps[:])

            def stats_chunk(c):
                sq = sqp.tile([P, KT, F], BF, tag="sq", name=f"sq{c}")
                nc.vector.tensor_tensor(sq[:], xts[c][:], xts[c][:], ALU.mult)
                Sx = pst.tile([1, F], FP, tag="sx", name=f"sx{c}")
                Sxx = pst.tile([1, F], FP, tag="sxx", name=f"sxx{c}")
                for kt in range(KT):
                    nc.tensor.matmul(
                        Sx[:],
                        lhsT=ones_t[:],
                        rhs=xts[c][:, kt, :],
                        start=(kt == 0),
                        stop=(kt == KT - 1),
                    )
                for kt in range(KT):
                    nc.tensor.matmul(
                        Sxx[:],
                        lhsT=ones_t[:],
                        rhs=sq[:, kt, :],
                        start=(kt == 0),
                        stop=(kt == KT - 1),
                    )
                # row math: -mu (bf16), mu^2, var, sigma (f32 + bf16)
                m2 = rows.tile([1, F], FP, tag="m2", name=f"m2{c}")
                nc.scalar.activation(
                    m2[:], Sx[:], func=AF.Square, bias=0.0, scale=1.0 / D
                )
                vt1 = rows.tile([1, F], FP, tag="vt1", name=f"vt1{c}")
                nc.vector.tensor_scalar(vt1[:], Sxx[:], 1.0 / D, None, ALU.mult)
                varx = rows.tile([1, F], FP, tag="varx", name=f"varx{c}")
                nc.vector.tensor_tensor(varx[:], vt1[:], m2[:], ALU.subtract)
                sgf = rows.tile([1, F], FP, tag="sgf", name=f"sgf{c}")
                nc.scalar.activation(
                    sgf[:], varx[:], func=AF.Sqrt, bias=eps_t[:], scale=1.0
                )
                # stack [-mu; sigma] on partitions 0/1 for one K=2 rank-1
                st2 = rows.tile([2, F], BF, tag="st2", name=f"st2{c}")
                nc.scalar.activation(
                    st2[0:1, :], Sx[:], func=AF.Copy, bias=0.0, scale=-1.0 / D
                )
                sgb = rows.tile([1, F], BF, tag="sgb", name=f"sgb{c}")
                nc.scalar.activation(
                    sgb[:], varx[:], func=AF.Sqrt, bias=eps_t[:], scale=1.0
                )
                nc.scalar.dma_start(st2[1:2, :], sgb[:])
                return st2, sgf

            def sig_transpose(c, sgf):
                # [1,512] sigma row -> r_sb[:, 4c:4c+4] columns via 4 tiny
                # PE transposes (rhs = 1x1 identity) + one PSUM reciprocal.
                pt = ptp.tile([P, CH], FP, tag="tp", name=f"sigT{c}")
                for u in range(CH):
                    nc.tensor.matmul(
                        pt[:, u : u + 1],
                        lhsT=sgf[0:1, u * P : (u + 1) * P],
                        rhs=idf[0:1, :],
                        is_transpose=True,
                        skip_group_check=True,
                        tile_position=(0, 0),
                    )
                nc.vector.reciprocal(r_sb[:, 4 * c : 4 * c + 4], pt[:])

            def q_chunk(c, st2):
                for e in range(ET):
                    ps = pmm.tile([P, F], FP, tag="ps", name=f"q{c}_{e}")
                    for kt in range(KT):
                        nc.tensor.matmul(
                            ps[:],
                            lhsT=wq_t[:, kt, e * P : (e + 1) * P],
                            rhs=xts[c][:, kt, :],
                            start=(kt == 0),
                            stop=False,
                        )
                    # K=2 rank-1: - mu (x) wqsum  +  sigma (x) q0
                    nc.tensor.matmul(
                        ps[:],
                        lhsT=wqq_t[0:2, e * P : (e + 1) * P],
                        rhs=st2[0:2, :],
                        start=False,
                        stop=True,
                    )
                    dst = qT[:, e, c * F : (c + 1) * F]
                    if e % 2 == 0:
                        nc.vector.tensor_copy(dst, ps[:])
                    else:
                        nc.scalar.copy(dst, ps[:])

            # ---- feed: PE order k0 s0 k1 T0 q0 s1 k2 T1 q1 s2 k3 T2 q2 s3 T3 q3
            k_chunk(0)
            nc.scalar.dma_start(wq_t[:], wq.rearrange("(p kt) e -> p kt e", kt=KT))
            nc.scalar.dma_start(wqq_t[:], wqq)
            sta = stats_chunk(0)
            k_chunk(1)
            sig_transpose(0, sta[1])
            q_chunk(0, sta[0])
            stb = stats_chunk(1)
            k_chunk(2)
            sig_transpose(1, stb[1])
            q_chunk(1, stb[0])
            stc = stats_chunk(2)
            k_chunk(3)
            sig_transpose(2, stc[1])
            q_chunk(2, stc[0])
            std = stats_chunk(3)
            sig_transpose(3, std[1])
            q_chunk(3, std[0])

            # ---------------- sim, exp (+z via accum), colsum, Y -------------
            cs_all = pcs.tile([P, F], FP)
            nc.vector.memset(cs_all[:], 0.0)
            ex_hist: list = [None, None]
            zrb_hist: list = [None, None]

            def colsum_mms(it):
                ex_t = ex_hist[it % 2]
                zrb_t = zrb_hist[it % 2]
                for jc in range(JC):
                    nc.tensor.matmul(
                        cs_all[32 * jc : 32 * jc + 1, :],
                        lhsT=zrb_t[:],
                        rhs=ex_t[:, jc * F : (jc + 1) * F],
                        start=(it == 0),
                        stop=(it == NT - 1),
                        skip_group_check=True,
                        tile_position=(0, 32 * jc),
                    )

            def sim_group(it, jc, ex, zpart):
                ps = pmm.tile([P, F], FP, tag="ps", name=f"sim{it}_{jc}")
                for et in range(ET):
                    nc.tensor.matmul(
                        ps[:],
                        lhsT=qT[:, et, it * P : (it + 1) * P],
                        rhs=kT[:, et, jc * F : (jc + 1) * F],
                        start=(et == 0),
                        stop=(et == ET - 1),
                    )
                nc.scalar.activation(
                    ex[:, jc * F : (jc + 1) * F],
                    ps[:],
                    func=AF.Exp,
                    bias=0.0,
                    scale=r_sb[:, it : it + 1],
                    accum_out=zpart[:, jc : jc + 1],
                )

            def y_group(g):
                c, mb, dh = g // 8, (g % 8) // 2, g % 2
                jt = 4 * c + mb
                psn = pyy.tile([P, F], FP, tag="py", name=f"y{g}")
                for kt in range(KT):
                    nc.tensor.matmul(
                        psn[:],
                        lhsT=mts[c][:, kt, mb * P : (mb + 1) * P],
                        rhs=w2_t[:, kt, dh * F : (dh + 1) * F],
                        start=(kt == 0),
                        stop=(kt == KT - 1),
                    )
                nc.vector.tensor_copy(Y[:, jt, dh * F : (dh + 1) * F], psn[:])

            for it in range(NT):
                ex = expp.tile([P, M], BF, tag="ex", name=f"ex{it}")
                zpart = zsp.tile([P, JC], FP, tag="zpt", name=f"zpt{it}")
                sim_group(it, 0, ex, zpart)
                sim_group(it, 1, ex, zpart)
                y_group(2 * it)
                sim_group(it, 2, ex, zpart)
                if it > 0:
                    colsum_mms(it - 1)
                sim_group(it, 3, ex, zpart)
                y_group(2 * it + 1)
                z = zsp.tile([P, 1], FP, tag="z", name=f"z{it}")
                nc.vector.tensor_reduce(z[:], zpart[:], axis=AX.X, op=ALU.add)
                zr = zsp.tile([P, 1], FP, tag="zr", name=f"zr{it}")
                nc.vector.reciprocal(zr[:], z[:])
                zrb = zrbp.tile([P, 1], BF, tag="zrb", name=f"zrb{it}")
                nc.vector.tensor_copy(zrb[:], zr[:])
                ex_hist[it % 2] = ex
                zrb_hist[it % 2] = zrb
            colsum_mms(NT - 1)

            # ---------------- tail -------------------------------------------
            # per jc-chunk: 1-row csum evac -> 4 transposes -> colsb columns
            # -> 4 scales (DVE-heavy; ScalarE copies are 2.5x slower) -> 4
            # single-tile stores, triggers alternating gpsimd/sync queues.
            colT = ptp.tile([P, NT], FP, tag="tp", name="colT")
            nc.vector.tensor_copy(csum_sb[:], cs_all[:])
            for jc in range(JC):
                for bb in range(4):
                    jt = 4 * jc + bb
                    nc.tensor.matmul(
                        colT[:, jt : jt + 1],
                        lhsT=csum_sb[32 * jc : 32 * jc + 1, bb * P : (bb + 1) * P],
                        rhs=idf[32 * jc : 32 * jc + 1, :],
                        is_transpose=True,
                        skip_group_check=True,
                        tile_position=(32 * jc, 0),
                    )
                nc.vector.tensor_copy(
                    colsb[:, 4 * jc : 4 * jc + 4], colT[:, 4 * jc : 4 * jc + 4]
                )
                for sh in range(2):
                    s_ = 2 * jc + sh
                    ob = obuf.tile([P, 2, D], BF, tag="ob", name=f"ob{s_}")
                    for h in range(2):
                        jt = 2 * s_ + h
                        csl = colsb[:, jt : jt + 1]
                        if jt % 4 == 3:
                            nc.scalar.mul(ob[:, h, :], Y[:, jt, :], csl)
                        elif jt % 4 == 1:
                            nc.gpsimd.tensor_scalar_mul(ob[:, h, :], Y[:, jt, :], csl)
                        else:
                            nc.vector.tensor_scalar_mul(ob[:, h, :], Y[:, jt, :], csl)
                    q = nc.gpsimd if s_ % 2 == 0 else nc.sync
                    q.dma_start(ov[:, 2 * s_ : 2 * s_ + 2, :], ob[:])

    nc.compile()
    return nc


_NC_CACHE = None


def _get_nc():
    global _NC_CACHE
    if _NC_CACHE is None:
        _NC_CACHE = _build()
    return _NC_CACHE


BF_NP = ml_dtypes.bfloat16


def _prep(inputs):
    ln_w = np.asarray(inputs["ln_w"], dtype=np.float32)
    ln_b = np.asarray(inputs["ln_b"], dtype=np.float32)
    Wq = np.asarray(inputs["Wq"], dtype=np.float32)
    Wkv = np.asarray(inputs["Wkv"], dtype=np.float32)
    Wout = np.asarray(inputs["Wout"], dtype=np.float32)

    def permute_rows(w):  # row (kt*P + p) -> row (p*KT + kt) for big packets
        ct = w.shape[0] // P
        return np.ascontiguousarray(
            w.reshape(ct, P, w.shape[1]).transpose(1, 0, 2).reshape(w.shape)
        )

    wq_f = Wq * (SCALE * ln_w)[:, None]
    wq_h = permute_rows(wq_f.astype(BF_NP))
    wk_h = permute_rows(np.ascontiguousarray(Wkv[:, :E]).astype(BF_NP))
    w2_h = permute_rows((Wkv[:, E:] @ Wout).astype(BF_NP))
    wqq_h = np.ascontiguousarray(
        np.stack([wq_f.sum(0), SCALE * (ln_b @ Wq)]).astype(BF_NP)
    )

    def t_chunks(a):  # [2048, 1024] -> [(c p kt), i'] = [4096, 512]
        at = np.ascontiguousarray(a.astype(BF_NP).T)          # [D, n]
        return np.ascontiguousarray(
            at.reshape(KT, P, CH, F).transpose(2, 1, 0, 3).reshape(CH * D, F)
        )

    xs = np.asarray(inputs["x"], dtype=np.float32)
    ms = np.asarray(inputs["media"], dtype=np.float32)
    shared = {"wq": wq_h, "wk": wk_h, "w2": w2_h, "wqq": wqq_h}
    return [
        dict(shared, xt=t_chunks(xs[b]), mt=t_chunks(ms[b])) for b in range(B)
    ]


def _unscramble(o):  # [2048, 1024] HBM rows p*16+jt -> position rows jt*128+p
    return np.ascontiguousarray(
        o.reshape(P, NT, D).transpose(1, 0, 2).reshape(M, D)
    ).astype(np.float32)


def _run(inputs, trace=False, **kw):
    nc = _get_nc()
    in_maps = _prep(inputs)
    res = run_bass_kernel_spmd(nc, in_maps, core_ids=list(range(B)), trace=trace, **kw)
    out = np.stack(
        [_unscramble(res.results[b]["out"]) for b in range(B)], axis=0
    )
    return out, res


def kernel(**inputs) -> np.ndarray:
    out, _ = _run(inputs, trace=False)
    return out
